# revision 24
# baseline (speedup 1.0000x reference)
"""Causal self-attention Bass/Trainium2 kernel.

Problem: B=4, T=2048, D=768, NH=12 heads (dh=64), fp32 I/O.

Sharding (8 NeuronCores, no collectives):
  core = b * 2 + hg  for batch b in 0..3, head-group hg in 0..1.
  Each core computes 6 heads (hg*6 .. hg*6+5) of one batch:
    Q/K/V projections for its heads, causal attention, and the partial
    output projection y_part = Z_part @ Wo_part (row-split contraction).
  Host sums the two partial outputs per batch and adds bo.

Per-core kernel layout (everything transposed so the contraction dim is
on partitions; host pre-transposes, which is free):
  xT  [768, 2048]          QT/KT [384, 2048] (pairs of heads per 128-row tile)
  V'  [2048, 6*65]         (ones column appended per head -> softmax sums)
  S^T [128k, 512q] blocks, P = exp(S/8) (no max subtraction: |logits| < 10),
  O'  = V'.T @ P^T accumulated over k tiles -> row 64 holds softmax sums.
  Normalize by broadcasting 1/sums, assemble Z^T, then y^T = Wo_sel @ Z.

v2 performance structure:
  - The two heads of a pair occupy partitions 0-63 / 64-127, so their
    K=64 QK matmuls land on disjoint PE row groups and run concurrently.
    QK blocks are emitted in runs of two so the A/B staircase never
    breaks against a full-row matmul.
  - Diagonal S^T blocks only compute the unmasked column range
    [off:512] in QK, exp, and PV (the masked strip is never read).
  - The last query tile's out-projection is pre-accumulated for pairs
    0/1 into SBUF (yacc) while pair 2's attention runs; the tail is six
    single matmuls whose PSUM is fused-added to yacc.
  - yT output and the reciprocal broadcast scratch are bf16.
"""

import numpy as np
import ml_dtypes

import concourse.bass as bass
from concourse import bacc
import concourse.mybir as mybir
import concourse.tile as tile
from concourse.bass_utils import run_bass_kernel_spmd

B, T, D, NH, DH = 4, 2048, 768, 12, 64
HPC = 6          # heads per core
NPAIR = 3        # head pairs per core
TQ = 512         # query tile (free dim of S^T blocks)
NQT = T // TQ    # 4
TKB = 128        # key tile (partition dim of S^T blocks)
NKT = T // TKB   # 16
KD = D // 128    # 6 contraction tiles for the projections
VW = DH + 1      # 65: V plus ones column

# Matmul/storage dtype: "bf16", "fp32", or "fp32r".
MM_MODE = "bf16"

_f32 = mybir.dt.float32
_bf16 = mybir.dt.bfloat16


def _dts():
    if MM_MODE == "bf16":
        return mybir.dt.bfloat16, ml_dtypes.bfloat16, False
    return _f32, np.float32, (MM_MODE == "fp32r")


def _build_program():
    ST_DT, _, use_r = _dts()

    def mm(ap):
        return ap.bitcast(mybir.dt.float32r) if use_r else ap

    nc = bacc.Bacc()
    # input layouts are partition-major and chunk-major so each tensor loads
    # with ONE fully-contiguous DMA (descriptor generation on the issue queue
    # costs ~600ns per DMA; many small DMAs serialize the head of the kernel)
    xT_d = nc.dram_tensor("xT", [NQT, 128, KD, TQ], ST_DT, kind="ExternalInput")
    wq_d = nc.dram_tensor("wqT", [128, KD, HPC * DH], ST_DT, kind="ExternalInput")
    wk_d = nc.dram_tensor("wkT", [128, KD, HPC * DH], ST_DT, kind="ExternalInput")
    wv_d = nc.dram_tensor("wvT", [128, KD, HPC * DH], ST_DT, kind="ExternalInput")
    wo_d = nc.dram_tensor("woT", [128, NPAIR, D], ST_DT, kind="ExternalInput")
    bq_d = nc.dram_tensor("bqT", [128, NPAIR], _f32, kind="ExternalInput")
    bk_d = nc.dram_tensor("bkT", [128, NPAIR], _f32, kind="ExternalInput")
    bvb_d = nc.dram_tensor("bvb", [HPC * VW], _f32, kind="ExternalInput")
    # DRAM scratch used to broadcast the per-column softmax sums across
    # partitions (SBUF->DRAM->stride-0 DMA back; DVE cannot cross partitions)
    scr_d = nc.dram_tensor("rscratch", [NPAIR, NQT, 2, TQ], _f32, kind="Internal")
    scr2_d = nc.dram_tensor("rscratch2", [NPAIR, NQT, 2, TQ], _bf16, kind="Internal")
    yT_d = nc.dram_tensor("yT", [KD, 128, T], _bf16, kind="ExternalOutput")

    with tile.TileContext(nc) as tc:
        with (
            tc.tile_pool(name="const", bufs=1) as const,
            tc.tile_pool(name="ptp", bufs=4) as ptp,
            tc.tile_pool(name="workp", bufs=3) as workp,
            tc.tile_pool(name="ps512", bufs=4, space="PSUM") as ps512,
            tc.tile_pool(name="ps1024", bufs=2, space="PSUM") as ps1024,
        ):
            # ---- constants / persistent tensors ----
            xT_sb = const.tile([128, KD, T], ST_DT)
            wq_sb = const.tile([128, KD, HPC * DH], ST_DT)
            wk_sb = const.tile([128, KD, HPC * DH], ST_DT)
            wv_sb = const.tile([128, KD, HPC * DH], ST_DT)
            wo_sb = const.tile([128, NPAIR, D], ST_DT)
            bq_sb = const.tile([128, NPAIR], _f32)
            bk_sb = const.tile([128, NPAIR], _f32)
            bvb_sb = const.tile([128, HPC * VW], _f32)
            qt_sb = const.tile([128, NPAIR, T], ST_DT)
            kt_sb = const.tile([128, NPAIR, T], ST_DT)
            v_sb = const.tile([128, NKT, HPC * VW], ST_DT)
            zt_sb = const.tile([128, NPAIR, T], ST_DT)
            # pairs-0/1 partial out-projection, staged per query tile
            yacc_sb = const.tile([128, KD, NQT, TQ], _bf16)

            # DMA order follows first-use: the pair-0 K/Q projections and the
            # first query tile's V blocks gate the start of attention, so
            # wk/wq/wv and the first xT chunk go first; later xT chunks land
            # while compute is already running.
            # spread the gating input loads across three issue queues so
            # descriptor generation happens in parallel
            xT_by_c = xT_sb.rearrange("p k (c q) -> p k c q", q=TQ)
            nc.sync.dma_start(out=wk_sb, in_=wk_d[:, :, :])
            nc.scalar.dma_start(out=wq_sb, in_=wq_d[:, :, :])
            nc.gpsimd.dma_start(out=xT_by_c[:, :, 0, :], in_=xT_d[0])
            nc.sync.dma_start(out=wv_sb, in_=wv_d[:, :, :])
            nc.scalar.dma_start(out=bq_sb, in_=bq_d[:, :])
            nc.sync.dma_start(out=bk_sb, in_=bk_d[:, :])
            for c in range(1, 4):
                nc.sync.dma_start(out=xT_by_c[:, :, c, :], in_=xT_d[c])
            nc.sync.dma_start(out=wo_sb, in_=wo_d[:, :, :])
            bvb_ap = bvb_d[:]
            bvb_bcast = bass.AP(
                tensor=bvb_ap.tensor, offset=bvb_ap.offset,
                ap=[[0, 128]] + list(bvb_ap.ap),
            )
            nc.gpsimd.dma_start(out=bvb_sb, in_=bvb_bcast)

            # ones column of V' (softmax denominator accumulator)
            v_by_head = v_sb.rearrange("p m (h c) -> p m h c", c=VW)
            nc.vector.memset(v_by_head[:, :, :, DH:VW], 1.0)

            # one triangular causal mask tile: keep (col j) >= (row k)
            tri_sb = const.tile([128, TKB], ST_DT, name="tri")
            nc.vector.memset(tri_sb, 1.0)
            nc.gpsimd.affine_select(
                out=tri_sb,
                in_=tri_sb,
                compare_op=mybir.AluOpType.is_ge,
                fill=0.0,
                base=0,
                pattern=[[1, TKB]],
                channel_multiplier=-1,
            )

            # PE warm-up: dense dummy matmuls with no DMA deps keep the
            # tensor engine busy through the input DMA ramp so the HAM
            # clock-gate is at 2.4 GHz when real work arrives.
            dummy_sb = const.tile([128, TQ], ST_DT, name="dummy")
            nc.vector.memset(dummy_sb, 0.0)
            ps_warm = ps512.tile([128, TQ], _f32, tag="ps512", name="ps_warm")
            for _ in range(24):
                nc.tensor.matmul(
                    ps_warm, lhsT=mm(dummy_sb[:, 0:128]), rhs=mm(dummy_sb),
                    start=True, stop=True,
                )

            # ---- interleaved projections / attention / out-projection ----
            # All projection and out-projection matmul groups are emitted as
            # unit closures through a filler queue so they execute inside the
            # (otherwise ACT-bound) attention loops: this fills PE idle slots
            # and keeps the HAM clock-gate at full speed.
            from collections import deque

            queue = deque()          # pending (key, closure) units
            pending = {}             # key -> remaining unit count in queue
            emitted = set()

            def qk_group_units(which, mt, nt):
                w_sb, b_sb, dest = (
                    (wq_sb, bq_sb, qt_sb) if which == "q" else (wk_sb, bk_sb, kt_sb)
                )
                state = {}
                units = []
                for kt in range(KD):
                    def u(kt=kt):
                        if kt == 0:
                            state["ps"] = ps512.tile(
                                [128, TQ], _f32, tag="ps512", name="psg")
                        nc.tensor.matmul(
                            state["ps"],
                            lhsT=mm(w_sb[:, kt, mt * 128 : (mt + 1) * 128]),
                            rhs=mm(xT_sb[:, kt, nt * TQ : (nt + 1) * TQ]),
                            start=(kt == 0), stop=(kt == KD - 1),
                        )
                    units.append(u)
                def fin():
                    nc.vector.tensor_tensor(
                        out=dest[:, mt, nt * TQ : (nt + 1) * TQ],
                        in0=state["ps"],
                        in1=b_sb[:, mt : mt + 1].to_broadcast((128, TQ)),
                        op=mybir.AluOpType.add,
                    )
                units.append(fin)
                return units

            def v_group_units(mt):
                # all three pairs at once: rhs N=384
                state = {}
                units = []
                for kt in range(KD):
                    def u(kt=kt):
                        if kt == 0:
                            state["ps"] = ps512.tile(
                                [128, HPC * DH], _f32, tag="ps512", name="psg")
                        nc.tensor.matmul(
                            state["ps"],
                            lhsT=mm(xT_sb[:, kt, mt * 128 : (mt + 1) * 128]),
                            rhs=mm(wv_sb[:, kt, :]),
                            start=(kt == 0), stop=(kt == KD - 1),
                        )
                    units.append(u)
                def fin():
                    nc.vector.tensor_tensor(
                        out=v_by_head[:, mt, :, 0:DH],
                        in0=state["ps"].rearrange("p (h c) -> p h c", c=DH),
                        in1=bvb_sb.rearrange("p (h c) -> p h c", c=VW)[:, :, 0:DH],
                        op=mybir.AluOpType.add,
                    )
                units.append(fin)
                return units

            def oacc01_group_units(mt, nt):
                # pairs 0/1 of the out-projection, staged to SBUF: these only
                # need zt of pairs 0/1, so they are filler supply for the
                # late-pair-1 / early-pair-2 stretch where the queue runs dry
                state = {}
                units = []
                for kt in range(2):
                    def u(kt=kt):
                        if kt == 0:
                            state["ps"] = ps512.tile(
                                [128, TQ], _f32, tag="ps512", name="psg")
                        nc.tensor.matmul(
                            state["ps"],
                            lhsT=mm(wo_sb[:, kt, mt * 128 : (mt + 1) * 128]),
                            rhs=mm(zt_sb[:, kt, nt * TQ : (nt + 1) * TQ]),
                            start=(kt == 0), stop=(kt == 1),
                        )
                    units.append(u)
                def fin():
                    nc.vector.tensor_copy(yacc_sb[:, mt, nt, :], state["ps"])
                units.append(fin)
                return units

            def ofin_group_units(mt, nt):
                state = {}
                units = []
                def u():
                    state["ps"] = ps512.tile(
                        [128, TQ], _f32, tag="ps512", name="psg")
                    nc.tensor.matmul(
                        state["ps"],
                        lhsT=mm(wo_sb[:, 2, mt * 128 : (mt + 1) * 128]),
                        rhs=mm(zt_sb[:, 2, nt * TQ : (nt + 1) * TQ]),
                        start=True, stop=True,
                    )
                units.append(u)
                def fin():
                    yt = workp.tile([128, TQ], _bf16, tag="yt", name="yt")
                    nc.vector.tensor_tensor(
                        out=yt, in0=state["ps"], in1=yacc_sb[:, mt, nt, :],
                        op=mybir.AluOpType.add,
                    )
                    nc.sync.dma_start(
                        out=yT_d[mt, :, nt * TQ : (nt + 1) * TQ], in_=yt)
                units.append(fin)
                return units

            def units_for(key):
                kind = key[0]
                if kind == "q" or kind == "k":
                    return qk_group_units(kind, key[1], key[2])
                if kind == "v":
                    return v_group_units(key[1])
                if kind == "oa":
                    return oacc01_group_units(key[1], key[2])
                return ofin_group_units(key[1], key[2])

            def push(key):
                if key in emitted:
                    return
                emitted.add(key)
                us = units_for(key)
                pending[key] = len(us)
                for u in us:
                    queue.append((key, u))

            def pop_unit():
                key, u = queue.popleft()
                u()
                pending[key] -= 1
                if pending[key] == 0:
                    del pending[key]

            # when the filler queue runs dry in the (ACT-bound) later pairs,
            # dependency-free dummy matmuls keep the PE HAM clock-gate warm so
            # the real matmuls around them stay at 2.4 GHz
            ham = {"budget": 0, "left": 150}

            def dummy_unit():
                pw = ps512.tile([128, TQ], _f32, tag="ps512", name="hamf")
                nc.tensor.matmul(
                    pw, lhsT=mm(dummy_sb[:, 0:128]), rhs=mm(dummy_sb),
                    start=True, stop=True,
                )

            def consume(n):
                for _ in range(n):
                    if queue:
                        pop_unit()
                    elif ham["budget"] > 0 and ham["left"] > 0:
                        ham["budget"] -= 1
                        ham["left"] -= 1
                        dummy_unit()

            def require(keys):
                # emit everything still queued for these groups right now
                for key in keys:
                    push(key)
                while any(pending.get(k, 0) > 0 for k in keys):
                    pop_unit()

            # queue pair-0 projections and all V in qt-demand order
            for nt in range(NQT):
                push(("k", 0, nt))
                push(("q", 0, nt))
                for mt in range(4 * nt, 4 * nt + 4):
                    push(("v", mt))

            # ---- attention per head pair ----
            for p in range(NPAIR):
                qA = qt_sb[0:64, p, :]
                qB = qt_sb[64:128, p, :]
                kA = kt_sb[0:64, p, :]
                kB = kt_sb[64:128, p, :]
                if p + 1 < NPAIR:  # queue next pair's first Q/K projections
                    for nt in range(2):
                        push(("k", p + 1, nt))
                        push(("q", p + 1, nt))
                # filler consumption rate per 2-block iteration: pair 0 has a
                # deep queue of forced work; pairs 1/2 must stretch their
                # remaining supply across the whole (ACT-bound) pair
                crate = 5 if p == 0 else 4
                for qt in range(NQT):
                    nk = 4 * (qt + 1)
                    if p + 1 < NPAIR and qt == 2:
                        # second half of the next pair's Q/K projections: kept
                        # back so filler supply lasts into the next pair
                        for nt in range(2, NQT):
                            push(("k", p + 1, nt))
                            push(("q", p + 1, nt))
                    require([("q", p, qt)])
                    oA = ps512.tile([128, TQ], _f32, tag="ps512", name="oA")
                    oB = ps512.tile([128, TQ], _f32, tag="ps512", name="oB")
                    qsl = slice(qt * TQ, (qt + 1) * TQ)
                    pts = [None] * nk

                    def off_of(kt):
                        return (kt - 4 * qt) * TKB if kt >= 4 * qt else 0

                    def emit_qk(kt):
                        # restricted to the unmasked column range [off:TQ];
                        # head A on partitions 0-63 / head B on 64-127 land on
                        # disjoint PE row groups and run concurrently
                        off = off_of(kt)
                        sab = ps1024.tile([128, 2, TQ], _f32, tag="sab", name="sab")
                        ksl = slice(kt * TKB, (kt + 1) * TKB)
                        cqsl = slice(qt * TQ + off, (qt + 1) * TQ)
                        nc.tensor.matmul(
                            sab[:, 0, off:], lhsT=mm(kA[:, ksl]), rhs=mm(qA[:, cqsl]),
                            start=True, stop=True,
                        )
                        nc.tensor.matmul(
                            sab[:, 1, off:], lhsT=mm(kB[:, ksl]), rhs=mm(qB[:, cqsl]),
                            start=True, stop=True,
                        )
                        pt = ptp.tile([128, 2, TQ], ST_DT, tag="pt", name="pt")
                        nc.scalar.activation(
                            out=pt[:, :, off:TQ], in_=sab[:, :, off:TQ],
                            func=mybir.ActivationFunctionType.Exp,
                            scale=0.125,
                        )
                        if kt >= 4 * qt:  # diagonal block: mask the 128-col
                            dsl = slice(off, off + TKB)  # triangle only
                            nc.vector.tensor_mul(
                                pt[:, 0, dsl], pt[:, 0, dsl], tri_sb)
                            nc.vector.tensor_mul(
                                pt[:, 1, dsl], pt[:, 1, dsl], tri_sb)
                        pts[kt] = pt

                    def emit_pv(kt):
                        # masked strip [0:off) never read; kt=0 (off=0)
                        # initializes the full PSUM width via start=True
                        off = off_of(kt)
                        st, sp = (kt == 0), (kt == nk - 1)
                        pt = pts[kt]
                        nc.tensor.matmul(
                            oA[0:VW, off:],
                            lhsT=mm(v_sb[:, kt, (2 * p) * VW : (2 * p + 1) * VW]),
                            rhs=mm(pt[:, 0, off:]),
                            start=st, stop=sp,
                        )
                        nc.tensor.matmul(
                            oB[0:VW, off:],
                            lhsT=mm(v_sb[:, kt, (2 * p + 1) * VW : (2 * p + 2) * VW]),
                            rhs=mm(pt[:, 1, off:]),
                            start=st, stop=sp,
                        )
                        pts[kt] = None

                    # software pipeline in runs of two blocks: the four QK
                    # matmuls stay adjacent in the PE queue (A/B staircase),
                    # PE two blocks ahead of ACT; filler units keep PE dense
                    for kt2 in range(0, nk, 2):
                        if kt2 == 2 and qt >= 1:
                            # out-projection fillers gated on the previous
                            # qt's zt chains, which have had a block run to
                            # finish: pair 1 stages oacc01 of its previous
                            # qt; pair 2 finishes ofin of its previous qt
                            if p == 1:
                                for mt in range(KD):
                                    push(("oa", mt, qt - 1))
                            elif p == 2:
                                if qt == 1:
                                    for mt in range(KD):
                                        push(("oa", mt, NQT - 1))
                                for mt in range(KD):
                                    push(("of", mt, qt - 1))
                        require([("k", p, kt2 // 4), ("k", p, (kt2 + 1) // 4)])
                        emit_qk(kt2)
                        emit_qk(kt2 + 1)
                        if kt2 >= 2:
                            require([("v", kt2 - 2), ("v", kt2 - 1)])
                            emit_pv(kt2 - 2)
                            emit_pv(kt2 - 1)
                        ham["budget"] = 2 if p >= 1 else 0
                        consume(crate)
                    require([("v", nk - 2), ("v", nk - 1)])
                    consume(2)
                    emit_pv(nk - 2)
                    consume(2)
                    emit_pv(nk - 1)

                    # stage O' to SBUF immediately (frees both PSUM banks;
                    # a DVE copy costs the same regardless of partition count)
                    cAB = workp.tile([65, 2, TQ], _f32, tag="cAB", name="cAB")
                    nc.vector.tensor_copy(cAB[:, 0, :], oA[0:VW, :])
                    if p == NPAIR - 1:
                        # ACT is idle near the end of the kernel: run the B
                        # copy there so both copies overlap (shorter chain)
                        nc.scalar.copy(cAB[:, 1, :], oB[0:VW, :])
                    else:
                        nc.vector.tensor_copy(cAB[:, 1, :], oB[0:VW, :])
                    # normalize by the accumulated softmax sums (row 64).
                    # Reshape the 2x512 sums through DRAM into [64,16] so the
                    # (multi-pass) DVE reciprocal runs 64-partition-parallel.
                    # All chain DMAs go through the (otherwise idle) gpsimd
                    # queue so they never wait behind input/output DMA issue.
                    nc.gpsimd.dma_start(
                        out=scr_d[p, qt].rearrange("a b -> (a b)"),
                        in_=cAB[64:65].rearrange("p a b -> p (a b)"))
                    sAB = workp.tile([64, 16], _f32, tag="sAB", name="sAB")
                    flat = scr_d[p, qt].rearrange("a b -> (a b)").rearrange(
                        "(p f) -> p f", p=64)
                    nc.gpsimd.dma_start(out=sAB, in_=flat)
                    rAB = workp.tile([64, 16], _bf16, tag="rAB", name="rAB")
                    with nc.allow_low_precision(
                        reason="zt itself is bf16; bf16 1/s adds no error"
                    ):
                        nc.vector.reciprocal(out=rAB, in_=sAB)
                    flat2 = scr2_d[p, qt].rearrange("a b -> (a b)").rearrange(
                        "(p f) -> p f", p=64)
                    nc.gpsimd.dma_start(out=flat2, in_=rAB)
                    rb = workp.tile([64, 2, TQ], _bf16, tag="rb", name="rb")
                    rAB2 = scr2_d[p, qt].rearrange("a b -> (a b)")
                    nc.gpsimd.dma_start(out=rb, in_=bass.AP(
                        tensor=rAB2.tensor, offset=rAB2.offset,
                        ap=[[0, 64]] + list(rAB2.ap)))
                    nc.vector.tensor_mul(
                        zt_sb[0:64, p, qsl], cAB[0:64, 0, :], rb[:, 0, :])
                    ztmp = workp.tile([64, TQ], ST_DT, tag="ztmp", name="ztmp")
                    nc.gpsimd.tensor_mul(ztmp, cAB[0:64, 1, :], rb[:, 1, :])
                    nc.gpsimd.dma_start(out=zt_sb[64:128, p, qsl], in_=ztmp)

            # tail: only the pair-2 matmul of the last query tile remains
            for mt in range(KD):
                push(("of", mt, NQT - 1))
            while queue:
                pop_unit()

    if not nc.is_finalized():
        nc.finalize()
    return nc


_CACHE = {}


def get_program():
    key = MM_MODE
    if key not in _CACHE:
        _CACHE[key] = _build_program()
    return _CACHE[key]


def make_in_maps(x, wq, bq, wk, bk, wv, bv, wo, bo):
    _, np_dt, _ = _dts()
    x, wq, bq, wk, bk, wv, bv, wo, bo = (
        np.asarray(a, dtype=np.float32) for a in (x, wq, bq, wk, bk, wv, bv, wo, bo)
    )
    in_maps = []
    for core in range(8):
        b, hg = core // 2, core % 2
        sl = slice(hg * HPC * DH, (hg + 1) * HPC * DH)
        # [NQT, 128, KD, TQ]: chunk-major, partition-major, contiguous per DMA
        xT = np.ascontiguousarray(
            x[b].T.astype(np_dt).reshape(KD, 128, NQT, TQ).transpose(2, 1, 0, 3))
        def wtile(w):  # [128, KD, 384]
            return np.ascontiguousarray(
                w[sl, :].T.astype(np_dt).reshape(KD, 128, HPC * DH).transpose(1, 0, 2))
        wqT, wkT, wvT = wtile(wq), wtile(wk), wtile(wv)
        woT = np.ascontiguousarray(
            wo[:, sl].T.astype(np_dt).reshape(NPAIR, 128, D).transpose(1, 0, 2))
        bqT = np.ascontiguousarray(bq[sl].reshape(NPAIR, 128).T)
        bkT = np.ascontiguousarray(bk[sl].reshape(NPAIR, 128).T)
        bvb = np.zeros((HPC, VW), np.float32)
        bvb[:, :DH] = bv[sl].reshape(HPC, DH)
        bvb[:, DH] = 1.0
        in_maps.append(
            dict(xT=xT, wqT=wqT, wkT=wkT, wvT=wvT, woT=woT,
                 bqT=bqT, bkT=bkT, bvb=bvb.reshape(-1))
        )
    return in_maps


def assemble_output(results, bo):
    y = np.zeros((B, T, D), np.float32)
    for core in range(8):
        y[core // 2] += results[core]["yT"].astype(np.float32).reshape(D, T).T
    y += np.asarray(bo, np.float32)[None, None, :]
    return y


def kernel(**inputs):
    nc = get_program()
    in_maps = make_in_maps(**inputs)
    res = run_bass_kernel_spmd(nc, in_maps, core_ids=list(range(8)))
    return assemble_output(res.results, inputs["bo"])


if __name__ == "__main__":
    nc = get_program()
    print("program built OK")
